# revision 1
# baseline (speedup 1.0000x reference)
"""CenterEmpiricalPriorMemory Trainium2 kernel (8 NeuronCores, SPMD).

Math (reference):
  retrieved  = slots[labels]                         [N, D]   gather
  seg_sum    = segment_sum(states, labels, C)        [C, D]
  bcounts    = histogram(labels, C)                  [C]
  mean       = seg_sum / max(bcounts, 1)
  ema        = 0.95*slots + 0.05*mean
  new_slots  = where(bcounts>0, where(counts<=0, mean, ema), slots)
  new_counts = counts + bcounts

Device strategy (per sharding hint):
  - Data-parallel over N: each of 8 cores gets N/8 = 16384 rows.
  - Per-core segment-sum via one-hot matmul on the TensorEngine in
    float32r (1 cyc/row; ~1e-4 rounding), accumulated over 128 row-tiles
    into PSUM [1024, 258] (256 dims + ones column for counts + pad to
    even free size, an fp32r ISA requirement).
  - One-hot [128, 1024] built on the VectorEngine: is_equal(iota, label).
  - ReduceScatter(add) over the 8 cores of the [1024, 258] partials;
    core i receives centers [128*i, 128*(i+1)) and applies the EMA /
    cold-start / passthrough update for its shard.
  - retrieved via dma_gather (GPSIMD indirect DMA) from the HBM slots
    table, 2048 rows per instruction.
Host assembles: concat retrieved shards (row order) and center shards.
"""
import sys

sys.path.insert(0, "/opt/trn_rl_repo")

import numpy as np

N_CORES = 8
N = 131072
C = 1024
D = 256
DP = D + 2          # ones column + pad column (fp32r needs even free dim)
N_LOC = N // N_CORES          # 16384
NT = N_LOC // 128             # 128 row-tiles per core
GCH = 2048                    # gather rows per dma_gather instruction
NG = N_LOC // GCH             # gather instructions per core
CSH = C // N_CORES            # 128 centers per core
MOMENTUM = 0.05

_cache = {}


def _build():
    if "nc" in _cache:
        return _cache["nc"]
    import concourse.bass as bass
    import concourse.bacc as bacc
    import concourse.mybir as mybir
    import concourse.tile as tile
    from concourse import library_config

    F32 = mybir.dt.float32
    F32R = mybir.dt.float32r
    I16 = mybir.dt.int16
    OP = mybir.AluOpType

    nc = bacc.Bacc("TRN2", target_bir_lowering=False, debug=False,
                   num_devices=N_CORES)

    xa_d = nc.dram_tensor("xa", [N_LOC, DP], F32, kind="ExternalInput")
    labt_d = nc.dram_tensor("labt", [128, NT], F32, kind="ExternalInput")
    iota_d = nc.dram_tensor("iota", [128, C], F32, kind="ExternalInput")
    slots_d = nc.dram_tensor("slots", [C, D], F32, kind="ExternalInput")
    gidx_d = nc.dram_tensor("gidx", [128, N_LOC // 16], I16, kind="ExternalInput")
    ssl_d = nc.dram_tensor("sslots", [CSH, D], F32, kind="ExternalInput")
    sct_d = nc.dram_tensor("scounts", [CSH, 1], F32, kind="ExternalInput")

    ret_d = nc.dram_tensor("retrieved", [N_LOC, D], F32, kind="ExternalOutput")
    nsl_d = nc.dram_tensor("nslots", [CSH, D], F32, kind="ExternalOutput")
    nct_d = nc.dram_tensor("ncounts", [CSH, 1], F32, kind="ExternalOutput")

    with tile.TileContext(nc) as tc:
        with (
            tc.tile_pool(name="const", bufs=1) as cpool,
            tc.tile_pool(name="work", bufs=4) as pool,
            tc.tile_pool(name="gpool", bufs=4) as gpool,
            tc.tile_pool(name="dram", bufs=1, space="DRAM") as dpool,
            tc.tile_pool(name="psum", bufs=1, space="PSUM") as psum,
        ):
            nc.gpsimd.load_library(library_config.mlp)

            iota_sb = cpool.tile([128, C], F32)
            nc.sync.dma_start(iota_sb[:], iota_d[:])
            labt_sb = cpool.tile([128, NT], F32)
            nc.sync.dma_start(labt_sb[:], labt_d[:])
            gidx_sb = cpool.tile([128, N_LOC // 16], I16)
            nc.sync.dma_start(gidx_sb[:], gidx_d[:])
            ss_sb = cpool.tile([128, D], F32)
            nc.sync.dma_start(ss_sb[:], ssl_d[:])
            sc_sb = cpool.tile([128, 1], F32)
            nc.sync.dma_start(sc_sb[:], sct_d[:])

            # ---- retrieved = slots[labels] via indirect gather DMA ----
            for g in range(NG):
                gd = gpool.tile([128, GCH // 128, D], F32, tag="gd", name=f"gd{g}")
                nc.gpsimd.dma_gather(
                    out_ap=gd[:],
                    in_ap=slots_d[:],
                    idxs_ap=gidx_sb[:, g * (GCH // 16):(g + 1) * (GCH // 16)],
                    num_idxs=GCH,
                    num_idxs_reg=GCH,
                    elem_size=D,
                    single_packet=False,
                )
                nc.sync.dma_start(
                    ret_d[g * GCH:(g + 1) * GCH, :].rearrange("(a p) d -> p a d", p=128),
                    gd[:],
                )

            # ---- per-core segment sum: one-hot matmul, PSUM accumulate ----
            acc = [psum.tile([128, DP], F32, tag=f"acc{cb}", name=f"acc{cb}")
                   for cb in range(8)]
            for t in range(NT):
                xa = pool.tile([128, DP], F32R, tag="xa", name=f"xa{t}")
                nc.gpsimd.dma_start(xa[:], xa_d[t * 128:(t + 1) * 128, :])
                oh = pool.tile([128, C], F32R, tag="oh", name=f"oh{t}")
                nc.vector.tensor_scalar(
                    oh[:], iota_sb[:], labt_sb[:, t:t + 1], None, OP.is_equal
                )
                for cb in range(8):
                    nc.tensor.matmul(
                        acc[cb][:],
                        oh[:, cb * 128:(cb + 1) * 128],
                        xa[:],
                        start=(t == 0),
                        stop=(t == NT - 1),
                    )

            seg_sb = pool.tile([128, 8 * DP], F32)
            for cb in range(8):
                nc.vector.tensor_copy(seg_sb[:, cb * DP:(cb + 1) * DP], acc[cb][:])

            seg_dram = dpool.tile([C, DP], F32)
            nc.sync.dma_start(
                seg_dram[:].rearrange("(cb p) e -> p cb e", p=128),
                seg_sb[:].rearrange("p (cb e) -> p cb e", cb=8),
            )

            rs_dram = dpool.tile([CSH, DP], F32)
            nc.gpsimd.collective_compute(
                "ReduceScatter",
                mybir.AluOpType.add,
                ins=[seg_dram.opt()],
                outs=[rs_dram.opt()],
                replica_groups=[list(range(N_CORES))],
            )

            rs_sb = pool.tile([128, DP], F32)
            nc.sync.dma_start(rs_sb[:], rs_dram[:])

            # ---- slot update for this core's 128-center shard ----
            seg = rs_sb[:, 0:D]
            bc = rs_sb[:, D:D + 1]
            f = pool.tile([128, 5], F32)   # [den, rden, cold, present, ncnt]
            nc.vector.tensor_scalar(f[:, 0:1], bc, 1.0, None, OP.max)
            nc.vector.reciprocal(f[:, 1:2], f[:, 0:1])
            nc.vector.tensor_scalar(f[:, 2:3], sc_sb[:], 0.0, None, OP.is_le)
            nc.vector.tensor_scalar(f[:, 3:4], bc, 0.0, None, OP.is_gt)
            nc.vector.tensor_tensor(f[:, 4:5], sc_sb[:], bc, OP.add)

            mean = pool.tile([128, D], F32)
            nc.vector.tensor_scalar(mean[:], seg, f[:, 1:2], None, OP.mult)
            # ema = 0.95*slots + 0.05*mean
            ema = pool.tile([128, D], F32)
            nc.vector.tensor_scalar(ema[:], ss_sb[:], 1.0 - MOMENTUM, None, OP.mult)
            t0 = pool.tile([128, D], F32)
            nc.vector.tensor_scalar(t0[:], mean[:], MOMENTUM, None, OP.mult)
            nc.vector.tensor_tensor(ema[:], ema[:], t0[:], OP.add)
            # cand = ema + cold*(mean-ema)
            nc.vector.tensor_tensor(t0[:], mean[:], ema[:], OP.subtract)
            nc.vector.tensor_scalar(t0[:], t0[:], f[:, 2:3], None, OP.mult)
            nc.vector.tensor_tensor(ema[:], ema[:], t0[:], OP.add)
            # new_slots = slots + present*(cand-slots)
            nc.vector.tensor_tensor(t0[:], ema[:], ss_sb[:], OP.subtract)
            nc.vector.tensor_scalar(t0[:], t0[:], f[:, 3:4], None, OP.mult)
            nsl_sb = pool.tile([128, D], F32)
            nc.vector.tensor_tensor(nsl_sb[:], ss_sb[:], t0[:], OP.add)

            nc.sync.dma_start(nsl_d[:], nsl_sb[:])
            nc.sync.dma_start(nct_d[:], f[:, 4:5])

    nc.compile()
    _cache["nc"] = nc
    return nc


def make_in_maps(states, slots, counts, center_labels):
    states = np.ascontiguousarray(np.asarray(states, dtype=np.float32))
    slots = np.ascontiguousarray(np.asarray(slots, dtype=np.float32))
    counts = np.asarray(counts, dtype=np.float32).reshape(C)
    labels = np.clip(np.asarray(center_labels).astype(np.int64), 0, C - 1)

    iota_np = np.ascontiguousarray(
        np.broadcast_to(np.arange(C, dtype=np.float32), (128, C))
    )
    j = np.arange(N_LOC)
    in_maps = []
    for i in range(N_CORES):
        st = states[i * N_LOC:(i + 1) * N_LOC]
        lb = labels[i * N_LOC:(i + 1) * N_LOC]
        xa = np.empty((N_LOC, DP), np.float32)
        xa[:, :D] = st
        xa[:, D] = 1.0
        xa[:, D + 1] = 0.0
        labt = np.ascontiguousarray(lb.reshape(NT, 128).T.astype(np.float32))
        gidx = np.zeros((128, N_LOC // 16), np.int16)
        gidx[j % 16, j // 16] = lb
        gidx = np.ascontiguousarray(np.tile(gidx[:16], (8, 1)))
        in_maps.append({
            "xa": xa,
            "labt": labt,
            "iota": iota_np,
            "slots": slots,
            "gidx": gidx,
            "sslots": np.ascontiguousarray(slots[i * CSH:(i + 1) * CSH]),
            "scounts": np.ascontiguousarray(counts[i * CSH:(i + 1) * CSH].reshape(CSH, 1)),
        })
    return in_maps


def assemble(results):
    retrieved = np.concatenate([np.asarray(r["retrieved"]) for r in results], 0)
    new_slots = np.concatenate([np.asarray(r["nslots"]) for r in results], 0)
    new_counts = np.concatenate(
        [np.asarray(r["ncounts"]).reshape(CSH) for r in results], 0
    )
    return retrieved, new_slots, new_counts


def kernel(states, slots, counts, center_labels):
    from concourse.bass_utils import run_bass_kernel_spmd

    nc = _build()
    in_maps = make_in_maps(states, slots, counts, center_labels)
    res = run_bass_kernel_spmd(nc, in_maps, list(range(N_CORES)))
    return assemble(res.results)


if __name__ == "__main__":
    rng = np.random.default_rng(0)
    states = rng.standard_normal((N, D)).astype(np.float32)
    labels = rng.integers(0, C, N).astype(np.int64)
    slots = rng.standard_normal((C, D)).astype(np.float32)
    counts = rng.integers(0, 3, C).astype(np.float32)
    r, ns, ncn = kernel(states=states, slots=slots, counts=counts,
                        center_labels=labels)
    print("shapes:", r.shape, ns.shape, ncn.shape)


# revision 11
# speedup vs baseline: 34770.9886x; 34770.9886x over previous
"""CenterEmpiricalPriorMemory Trainium2 kernel (8 NeuronCores, SPMD).

Math (reference):
  retrieved  = slots[labels]                         [N, D]   gather
  seg_sum    = segment_sum(states, labels, C)        [C, D]
  bcounts    = histogram(labels, C)                  [C]
  mean       = seg_sum / max(bcounts, 1)
  ema        = 0.95*slots + 0.05*mean
  new_slots  = where(bcounts>0, where(counts<=0, mean, ema), slots)
  new_counts = counts + bcounts

Device strategy (per sharding hint):
  - Data-parallel over N: each of 8 cores gets N/8 = 16384 rows.
  - Per-core segment-sum via one-hot matmul on the TensorEngine in
    float32r (1 cyc/row; ~1e-4 rounding), accumulated over 128 row-tiles
    into PSUM [1024, 258] (256 dims + ones column for counts + pad to
    even free size, an fp32r ISA requirement).
  - One-hot [128, 1024] built on the VectorEngine: is_equal(iota, label).
  - ReduceScatter(add) over the 8 cores of the [1024, 258] partials;
    core i receives centers [128*i, 128*(i+1)) and applies the EMA /
    cold-start / passthrough update for its shard.
  - retrieved via dma_gather (GPSIMD indirect DMA) from the HBM slots
    table, 2048 rows per instruction.
Host assembles: concat retrieved shards (row order) and center shards.
"""
import sys

sys.path.insert(0, "/opt/trn_rl_repo")

import numpy as np

N_CORES = 8
N = 131072
C = 1024
D = 256
DP = D + 2          # ones column + pad column (fp32r needs even free dim)
N_LOC = N // N_CORES          # 16384
NT = N_LOC // 128             # 128 row-tiles per core
GCH = 512                    # gather rows per dma_gather instruction
NG = N_LOC // GCH             # gather instructions per core
CSH = C // N_CORES            # 128 centers per core
MOMENTUM = 0.05

_cache = {}


def _build(collective=True):
    key = ("nc", collective)
    if key in _cache:
        return _cache[key]
    import concourse.bass as bass
    import concourse.bacc as bacc
    import concourse.mybir as mybir
    import concourse.tile as tile
    from concourse import library_config

    F32 = mybir.dt.float32
    F32R = mybir.dt.float32r
    I16 = mybir.dt.int16
    OP = mybir.AluOpType

    nc = bacc.Bacc("TRN2", target_bir_lowering=False, debug=False,
                   num_devices=N_CORES)

    xa_d = nc.dram_tensor("xa", [N_LOC, DP], F32, kind="ExternalInput")
    labt_d = nc.dram_tensor("labt", [128, NT], F32, kind="ExternalInput")
    iota_d = nc.dram_tensor("iota", [128, C], F32, kind="ExternalInput")
    slots_d = nc.dram_tensor("slots", [C, D], F32, kind="ExternalInput")
    gidx_d = nc.dram_tensor("gidx", [128, N_LOC // 16], I16, kind="ExternalInput")
    ssl_d = nc.dram_tensor("sslots", [CSH, D], F32, kind="ExternalInput")
    sct_d = nc.dram_tensor("scounts", [CSH, 1], F32, kind="ExternalInput")

    ret_d = nc.dram_tensor("retrieved", [N_LOC, D], F32, kind="ExternalOutput")
    nsl_d = nc.dram_tensor("nslots", [CSH, D], F32, kind="ExternalOutput")
    nct_d = nc.dram_tensor("ncounts", [CSH, 1], F32, kind="ExternalOutput")

    with tile.TileContext(nc) as tc:
        with (
            tc.tile_pool(name="const", bufs=1) as cpool,
            tc.tile_pool(name="work", bufs=4) as pool,
            tc.tile_pool(name="gpool", bufs=4) as gpool,
            tc.tile_pool(name="dram", bufs=1, space="DRAM") as dpool,
            tc.tile_pool(name="psum", bufs=1, space="PSUM") as psum,
        ):
            nc.gpsimd.load_library(library_config.mlp)

            iota_sb = cpool.tile([128, C], F32)
            nc.sync.dma_start(iota_sb[:], iota_d[:])
            labt_sb = cpool.tile([128, NT], F32)
            nc.sync.dma_start(labt_sb[:], labt_d[:])
            gidx_sb = cpool.tile([128, N_LOC // 16], I16)
            nc.sync.dma_start(gidx_sb[:], gidx_d[:])
            ss_sb = cpool.tile([128, D], F32)
            nc.sync.dma_start(ss_sb[:], ssl_d[:])
            sc_sb = cpool.tile([128, 1], F32)
            nc.sync.dma_start(sc_sb[:], sct_d[:])

            # retrieved = slots[labels] via indirect gather DMA; one chunk
            # is emitted every couple of matmul blocks so the gather traffic
            # spreads across the whole span instead of starving the xa loads.
            def emit_gather(g):
                gd = gpool.tile([128, GCH // 128, D], F32, tag="gd", name=f"gd{g}")
                nc.gpsimd.dma_gather(
                    out_ap=gd[:],
                    in_ap=slots_d[:],
                    idxs_ap=gidx_sb[:, g * (GCH // 16):(g + 1) * (GCH // 16)],
                    num_idxs=GCH,
                    num_idxs_reg=GCH,
                    elem_size=D,
                    single_packet=False,
                )
                nc.sync.dma_start(
                    ret_d[g * GCH:(g + 1) * GCH, :].rearrange("(a p) d -> p a d", p=128),
                    gd[:],
                )

            # ---- per-core segment sum: one-hot matmul, PSUM accumulate ----
            acc = [psum.tile([128, DP], F32, tag=f"acc{cb}", name=f"acc{cb}")
                   for cb in range(8)]
            XB = 8                       # row-tiles per batched load
            NBLK = NT // XB
            gathers_per_blk = NG / NBLK
            g_emitted = 0
            for blk in range(NT // XB):
                while g_emitted < min(NG, int((blk + 1) * gathers_per_blk)):
                    emit_gather(g_emitted)
                    g_emitted += 1
                xab = pool.tile([128, XB, DP], F32R, tag="xa", name=f"xa{blk}", bufs=6)
                nc.gpsimd.dma_start(
                    xab[:],
                    xa_d[blk * XB * 128:(blk + 1) * XB * 128, :]
                    .rearrange("(a p) e -> p a e", p=128),
                )
                for a in range(XB):
                    t = blk * XB + a
                    oh = pool.tile([128, C], F32R, tag="oh", name=f"oh{t}")
                    nc.vector.tensor_scalar(
                        oh[:], iota_sb[:], labt_sb[:, t:t + 1], None, OP.is_equal
                    )
                    for cb in range(8):
                        nc.tensor.matmul(
                            acc[cb][:],
                            oh[:, cb * 128:(cb + 1) * 128],
                            xab[:, a, :],
                            start=(t == 0),
                            stop=(t == NT - 1),
                        )

            seg_sb = pool.tile([128, 8 * DP], F32)
            for cb in range(8):
                nc.vector.tensor_copy(seg_sb[:, cb * DP:(cb + 1) * DP], acc[cb][:])

            seg_dram = dpool.tile([C, DP], F32)
            nc.sync.dma_start(
                seg_dram[:].rearrange("(cb p) e -> p cb e", p=128),
                seg_sb[:].rearrange("p (cb e) -> p cb e", cb=8),
            )

            rs_dram = dpool.tile([CSH, DP], F32)
            if collective:
                nc.gpsimd.collective_compute(
                    "ReduceScatter",
                    mybir.AluOpType.add,
                    ins=[seg_dram.opt()],
                    outs=[rs_dram.opt()],
                    replica_groups=[list(range(N_CORES))],
                )
            else:
                nc.sync.dma_start(rs_dram[:], seg_dram[0:CSH, :])

            rs_sb = pool.tile([128, DP], F32)
            nc.sync.dma_start(rs_sb[:], rs_dram[:])

            # ---- slot update for this core's 128-center shard ----
            seg = rs_sb[:, 0:D]
            bc = rs_sb[:, D:D + 1]
            f = pool.tile([128, 5], F32)   # [den, rden, cold, present, ncnt]
            nc.vector.tensor_scalar(f[:, 0:1], bc, 1.0, None, OP.max)
            nc.vector.reciprocal(f[:, 1:2], f[:, 0:1])
            nc.vector.tensor_scalar(f[:, 2:3], sc_sb[:], 0.0, None, OP.is_le)
            nc.vector.tensor_scalar(f[:, 3:4], bc, 0.0, None, OP.is_gt)
            nc.vector.tensor_tensor(f[:, 4:5], sc_sb[:], bc, OP.add)

            mean = pool.tile([128, D], F32)
            nc.vector.tensor_scalar(mean[:], seg, f[:, 1:2], None, OP.mult)
            # ema = 0.95*slots + 0.05*mean
            ema = pool.tile([128, D], F32)
            nc.vector.tensor_scalar(ema[:], ss_sb[:], 1.0 - MOMENTUM, None, OP.mult)
            t0 = pool.tile([128, D], F32)
            nc.vector.tensor_scalar(t0[:], mean[:], MOMENTUM, None, OP.mult)
            nc.vector.tensor_tensor(ema[:], ema[:], t0[:], OP.add)
            # cand = ema + cold*(mean-ema)
            nc.vector.tensor_tensor(t0[:], mean[:], ema[:], OP.subtract)
            nc.vector.tensor_scalar(t0[:], t0[:], f[:, 2:3], None, OP.mult)
            nc.vector.tensor_tensor(ema[:], ema[:], t0[:], OP.add)
            # new_slots = slots + present*(cand-slots)
            nc.vector.tensor_tensor(t0[:], ema[:], ss_sb[:], OP.subtract)
            nc.vector.tensor_scalar(t0[:], t0[:], f[:, 3:4], None, OP.mult)
            nsl_sb = pool.tile([128, D], F32)
            nc.vector.tensor_tensor(nsl_sb[:], ss_sb[:], t0[:], OP.add)

            nc.sync.dma_start(nsl_d[:], nsl_sb[:])
            nc.sync.dma_start(nct_d[:], f[:, 4:5])

    nc.compile()
    _cache[key] = nc
    return nc


def make_in_maps(states, slots, counts, center_labels):
    states = np.ascontiguousarray(np.asarray(states, dtype=np.float32))
    slots = np.ascontiguousarray(np.asarray(slots, dtype=np.float32))
    counts = np.asarray(counts, dtype=np.float32).reshape(C)
    labels = np.clip(np.asarray(center_labels).astype(np.int64), 0, C - 1)

    iota_np = np.ascontiguousarray(
        np.broadcast_to(np.arange(C, dtype=np.float32), (128, C))
    )
    j = np.arange(N_LOC)
    in_maps = []
    for i in range(N_CORES):
        st = states[i * N_LOC:(i + 1) * N_LOC]
        lb = labels[i * N_LOC:(i + 1) * N_LOC]
        xa = np.empty((N_LOC, DP), np.float32)
        xa[:, :D] = st
        xa[:, D] = 1.0
        xa[:, D + 1] = 0.0
        labt = np.ascontiguousarray(lb.reshape(NT, 128).T.astype(np.float32))
        gidx = np.zeros((128, N_LOC // 16), np.int16)
        gidx[j % 16, j // 16] = lb
        gidx = np.ascontiguousarray(np.tile(gidx[:16], (8, 1)))
        in_maps.append({
            "xa": xa,
            "labt": labt,
            "iota": iota_np,
            "slots": slots,
            "gidx": gidx,
            "sslots": np.ascontiguousarray(slots[i * CSH:(i + 1) * CSH]),
            "scounts": np.ascontiguousarray(counts[i * CSH:(i + 1) * CSH].reshape(CSH, 1)),
        })
    return in_maps


def assemble(results):
    retrieved = np.concatenate([np.asarray(r["retrieved"]) for r in results], 0)
    new_slots = np.concatenate([np.asarray(r["nslots"]) for r in results], 0)
    new_counts = np.concatenate(
        [np.asarray(r["ncounts"]).reshape(CSH) for r in results], 0
    )
    return retrieved, new_slots, new_counts


def kernel(states, slots, counts, center_labels):
    from concourse.bass_utils import run_bass_kernel_spmd

    nc = _build()
    in_maps = make_in_maps(states, slots, counts, center_labels)
    res = run_bass_kernel_spmd(nc, in_maps, list(range(N_CORES)))
    return assemble(res.results)


if __name__ == "__main__":
    rng = np.random.default_rng(0)
    states = rng.standard_normal((N, D)).astype(np.float32)
    labels = rng.integers(0, C, N).astype(np.int64)
    slots = rng.standard_normal((C, D)).astype(np.float32)
    counts = rng.integers(0, 3, C).astype(np.float32)
    r, ns, ncn = kernel(states=states, slots=slots, counts=counts,
                        center_labels=labels)
    print("shapes:", r.shape, ns.shape, ncn.shape)
